# revision 30
# baseline (speedup 1.0000x reference)
"""Trainium2 Bass kernel for nn_CrossModalAttention (M=8, D=256, B=8192).

Math restructuring (seq_len=1 MHA => out_proj(V_proj(x_t)) per (s,t) pair):
  hid[s]   = relu( W1x[s] @ x_s + sum_{t!=s} G[s,t] @ x_t + b1eff[s] )
  fused[s] = W2[s] @ hid[s] + b2[s]
  ch[s]    = relu( Wc1q @ rq + Wcf2[s] @ hid[s] + cb[s] )   Wcf2 = Wc1f @ W2[s]
  score[s] = sigmoid(wc2 . ch[s] + bc2)
  out      = mean_s fused[s] * score[s]
where G[s,t] = (W1c[s]/7) @ Wo[s,t] @ Wv[s,t] is folded on the HOST
(weight-only preprocessing), so the device runs a single dense pipeline.

Sharding: 8 cores = 4 batch shards x 2 modality groups. Core (g, i) handles
source modalities [4g..4g+3] for batch rows [i*2048, (i+1)*2048). All
activations are feature-major [feature, batch] in SBUF; weights ship as
pre-transposed bf16 lhsT blocks. The device loop is software-pipelined over
source modalities so the PE issues matmuls back-to-back while ACT/DVE handle
evictions in the shadow.
"""

import os
import sys
import types

import numpy as np
import ml_dtypes

# ---------------------------------------------------------------------------
# environment / concourse import
# ---------------------------------------------------------------------------
try:
    import concourse.bass as bass
except ImportError:  # pragma: no cover
    for p in ("/opt/trn_rl_repo", "/root/.axon_site/_ro/trn_rl_repo"):
        if os.path.isdir(p) and p not in sys.path:
            sys.path.insert(0, p)
    import concourse.bass as bass

import concourse.mybir as mybir
import concourse.tile as tile
from concourse.bass_utils import run_bass_kernel_spmd
from concourse.tile_sem_assignment import N_PROCS
from concourse.vector_clock import ScopedClock, VectorClock

F32 = mybir.dt.float32
BF16 = mybir.dt.bfloat16
FP8 = mybir.dt.float8e4
NP_BF16 = ml_dtypes.bfloat16
NP_FP8 = ml_dtypes.float8_e4m3
AFT = mybir.ActivationFunctionType
DR = mybir.MatmulPerfMode.DoubleRow

SX = 16.0      # fp8 scale on x
SG = 256.0     # fp8 scale on G
SH = SX * SG   # PSUM scale of the hid accumulation

# module-level knobs (test.py pokes these)
TRACE = False
USE_F32R = True  # unused; kept for test.py compat
LAST = {}

P = 128          # partitions
M = 8            # modalities
D = 256          # embedding dim
B = 8192         # batch
SM = 4           # source modalities per core
NB = 4           # batch tiles per core
TB = 512         # batch tile size (per-core batch = NB*TB = 2048)
BC = NB * TB

_MAX_WAITS = 1   # this walrus build supports one sync-wait per instruction


# ---------------------------------------------------------------------------
# walrus single-wait workaround: split multi-wait instructions
# ---------------------------------------------------------------------------
def _patched_drain_and_barrier(self, tick_clock, wait_clock):
    gc = tick_clock.global_clock
    for p in range(N_PROCS):
        t = gc[p]
        if t <= 0:
            continue
        sub = VectorClock([t if q == p else 0 for q in range(N_PROCS)])
        nop_inst = self.nc.sync.nop(nofuse=True)
        wait_clock.add_sem_waits(nop_inst.ins, ScopedClock({None: sub}))
    self.nc.sync.drain()
    self.nc.all_engine_barrier()
    assert self.sems is not None
    popped = self.nc._tile_sem_poison_stack.pop()
    assert popped is self._sem_poison
    self.nc.clear_and_free_semaphores(list(self.sems.allocated().values()))
    self.nc.all_engine_barrier()


_orig_commit_and_lower = None


def _patched_commit_and_lower(self, inst, original_block, old_bb_map, bb_to_exit_bb):
    si = getattr(inst, "sync_info", None)
    if (
        si is not None
        and si.on_wait
        and len(si.on_wait) > _MAX_WAITS
        and inst.engine != mybir.EngineType.Unassigned
    ):
        waits = list(si.on_wait)
        keep = waits[-_MAX_WAITS:]
        for w in waits[:-_MAX_WAITS]:
            nop = mybir.InstNoOp(
                name=self.nc.get_next_instruction_name(),
                sync_info=mybir.SyncInfo(on_wait=[w], on_update=[]),
                bass_nofuse=True,
                engine=inst.engine,
            )
            self._commit_instruction(nop)
        inst.sync_info = mybir.SyncInfo(on_wait=keep, on_update=list(si.on_update))
    return _orig_commit_and_lower(self, inst, original_block, old_bb_map, bb_to_exit_bb)


def _install_patches():
    global _orig_commit_and_lower
    if _orig_commit_and_lower is None:
        _orig_commit_and_lower = tile.TileContext._commit_and_lower
        tile.TileContext._drain_and_barrier = _patched_drain_and_barrier
        tile.TileContext._commit_and_lower = _patched_commit_and_lower


# ---------------------------------------------------------------------------
# optional NTFF profile hook (for HW exec-time measurement; safe no-op on fail)
# ---------------------------------------------------------------------------
def _install_ntff_hook():
    try:
        import antenv

        if "antenv.axon_hooks" in sys.modules:
            return True
        mod = types.ModuleType("antenv.axon_hooks")
        mod._hook = None
        mod.set_axon_ntff_profile_hook = lambda h: setattr(mod, "_hook", h)
        mod.get_axon_ntff_profile_hook = lambda: mod._hook
        sys.modules["antenv.axon_hooks"] = mod
        antenv.axon_hooks = mod
        from trn_agent_boot.trn_boot import _ntff_profile_via_ctypes

        hook = _ntff_profile_via_ctypes("/opt/axon/libaxon_pjrt.so")
        mod.set_axon_ntff_profile_hook(hook)
        return hook is not None
    except Exception:
        return False


# ---------------------------------------------------------------------------
# device program
# ---------------------------------------------------------------------------
_NC = None


def _build_nc():
    nc = bass.Bass()

    # inputs (per-core shards; same shapes on every core)
    # xd: bf16 x of the core's own 4 source modalities (diagonal matmuls)
    xd = nc.dram_tensor("xd", [NB, P, SM, 2, TB], BF16, kind="ExternalInput")
    # x8: fp8(e4m3) x of all 8 modalities, scaled by SX (cross matmuls)
    x8 = nc.dram_tensor("x8", [NB, P, M, 2, TB], FP8, kind="ExternalInput")
    rqT = nc.dram_tensor("rqT", [NB, P, 2, TB], BF16, kind="ExternalInput")
    # diag hid weights (W1x * SH): [p(d-in-chunk), sp, dc, jc, j']
    w1x = nc.dram_tensor("w1x", [P, SM, 2, 2, P], BF16, kind="ExternalInput")
    # cross hid weights G*SG as fp8 hi/lo pair: [p(d), sp, ti, dc, jc, j']
    g8h = nc.dram_tensor("g8h", [P, SM, M - 1, 2, 2, P], FP8,
                         kind="ExternalInput")
    g8l = nc.dram_tensor("g8l", [P, SM, M - 1, 2, 2, P], FP8,
                         kind="ExternalInput")
    # fused weights: [p(j-in-chunk), sp, jc, oc, o']
    w2w = nc.dram_tensor("w2w", [P, SM, 2, 2, P], BF16, kind="ExternalInput")
    # controller hid weights (Wc1f@W2): [p(j-in-chunk), sp, jc_in, jc_out, j'']
    wcf = nc.dram_tensor("wcf", [P, SM, 2, 2, P], BF16, kind="ExternalInput")
    # controller query weights: [p(d-in-chunk), dc, jc, j']
    wcq = nc.dram_tensor("wcq", [P, 2, 2, P], BF16, kind="ExternalInput")
    # column-replicated wc2: [p(j-in-chunk), jc, col]
    wc2r = nc.dram_tensor("wc2r", [P, 2, P], BF16, kind="ExternalInput")
    # f32 per-partition constants: [:,0:8] b1eff (sp,jc), [:,8:16] b2 (sp,oc),
    # [:,16:24] cb (sp,jc), [:,24] bc2
    smalls = nc.dram_tensor("smalls", [P, 25], F32, kind="ExternalInput")
    # identity matrix (lhsT) used to add rqp into the ch PSUM group
    ident = nc.dram_tensor("ident", [P, P], BF16, kind="ExternalInput")
    outT = nc.dram_tensor("outT", [NB, 2, P, TB], BF16, kind="ExternalOutput")

    mm = nc.tensor.matmul
    alu = mybir.AluOpType

    with tile.TileContext(nc) as tc:
        with (
            tc.tile_pool(name="const", bufs=1) as cpool,
            tc.tile_pool(name="xpool", bufs=2) as xpool,
            tc.tile_pool(name="x8pool", bufs=2) as x8pool,
            tc.tile_pool(name="rqpool", bufs=2) as rqpool,
            tc.tile_pool(name="rqppool", bufs=2) as rqppool,
            tc.tile_pool(name="hidpool", bufs=3) as hidpool,
            tc.tile_pool(name="fpool", bufs=2) as fpool,
            tc.tile_pool(name="chpool", bufs=2) as chpool,
            tc.tile_pool(name="scpool", bufs=2) as scpool,
            tc.tile_pool(name="gfpool", bufs=2) as gfpool,
            tc.tile_pool(name="accpool", bufs=2) as accpool,
            tc.tile_pool(name="psH", bufs=3, space="PSUM") as psH,
            tc.tile_pool(name="psF", bufs=2, space="PSUM") as psF,
            tc.tile_pool(name="psS", bufs=1, space="PSUM") as psS,
        ):
            # ---- resident constants (spread across DMA queues so the
            # startup transfers run on parallel DMA engines) ----
            sm_sb = cpool.tile([P, 25], F32, tag="smalls")
            nc.sync.dma_start(sm_sb[:], smalls[:])
            wcq_sb = cpool.tile([P, 2, 2, P], BF16, tag="wcq")
            nc.sync.dma_start(wcq_sb[:], wcq[:])
            id_sb = cpool.tile([P, P], BF16, tag="ident")
            nc.sync.dma_start(id_sb[:], ident[:])
            rqt0 = rqpool.tile([P, 2, TB], BF16, tag="rq")
            nc.sync.dma_start(rqt0[:], rqT[0])
            wc2_sb = cpool.tile([P, 2, P], BF16, tag="wc2r")
            nc.sync.dma_start(wc2_sb[:], wc2r[:])

            # PE warm-up: dummy matmuls on already-arriving tiles keep the
            # tensor engine busy through the startup DMA window so the HAM
            # clock ramps to 2.4 GHz before real work starts.
            ps_warm = psS.tile([P, TB], F32, tag="psS", name="ps_warm")
            for w in range(28):
                mm(ps_warm[:], wcq_sb[:, 0, 0, :], wcq_sb[:],
                   start=True, stop=True, skip_group_check=True)

            xd0 = xpool.tile([P, SM, 2, TB], BF16, tag="xd")
            nc.gpsimd.dma_start(xd0[:], xd[0])
            x80 = x8pool.tile([P, M, 2, TB], FP8, tag="x8")
            nc.gpsimd.dma_start(x80[:], x8[0])

            w1x_sb = cpool.tile([P, SM, 2, 2, P], BF16, tag="w1x")
            nc.scalar.dma_start(w1x_sb[:], w1x[:])
            g8h_sb = cpool.tile([P, SM, M - 1, 2, 2, P], FP8, tag="g8h")
            g8l_sb = cpool.tile([P, SM, M - 1, 2, 2, P], FP8, tag="g8l")
            nc.scalar.dma_start(g8h_sb[:, 0], g8h[:, 0])
            nc.scalar.dma_start(g8l_sb[:, 0], g8l[:, 0])

            w2_sb = cpool.tile([P, SM, 2, 2, P], BF16, tag="w2w")
            nc.sync.dma_start(w2_sb[:], w2w[:])
            wcf_sb = cpool.tile([P, SM, 2, 2, P], BF16, tag="wcf")
            nc.sync.dma_start(wcf_sb[:], wcf[:])
            for sp in range(1, SM):
                nc.scalar.dma_start(g8h_sb[:, sp], g8h[:, sp])
                nc.sync.dma_start(g8l_sb[:, sp], g8l[:, sp])

            def b1_ap(sp, jc):
                return sm_sb[:, sp * 2 + jc:sp * 2 + jc + 1]

            def b2_ap(sp, oc):
                return sm_sb[:, 8 + sp * 2 + oc:8 + sp * 2 + oc + 1]

            def cb_ap(sp, jc):
                return sm_sb[:, 16 + sp * 2 + jc:16 + sp * 2 + jc + 1]

            def bc2_ap():
                return sm_sb[:, 24:25]

            NSLOT = NB * SM
            xds = [xd0]
            x8s = [x80]
            rqts = [rqt0]
            state = {}

            def hid_mms(k):
                # x8 slots are host-permuted to [own 4 mods, other 4], so the
                # cross-target slots for local source sp are all t != sp,
                # identically on every core.
                nb, sp = divmod(k, SM)
                xdt = xds[nb]
                x8t = x8s[nb]
                ps = [psH.tile([P, TB], F32, tag="psH", name=f"psh{k}_{j}")
                      for j in range(2)]
                tlist = [t for t in range(M) if t != sp]
                for jc in range(2):
                    # diagonal W1x (bf16, pre-scaled by SH)
                    for dc in range(2):
                        mm(ps[jc][:], w1x_sb[:, sp, dc, jc, :],
                           xdt[:, sp, dc, :], start=(dc == 0), stop=False)
                    # cross G (fp8 hi+lo, DoubleRow over the dc pair)
                    i = 0
                    n = 2 * (M - 1)
                    for ti, t in enumerate(tlist):
                        for gsb in (g8h_sb, g8l_sb):
                            mm(ps[jc][:], gsb[:, sp, ti, :, jc, :],
                               x8t[:, t, :, :], start=False,
                               stop=(i == n - 1), perf_mode=DR,
                               skip_group_check=True)
                            i += 1
                state[("psh", k)] = ps

            def hid_ev_act(k):
                nb, sp = divmod(k, SM)
                ps = state[("psh", k)]
                hid = hidpool.tile([P, 2, TB], BF16, tag="hid")
                nc.scalar.activation(hid[:, 0, :], ps[0][:], AFT.Relu,
                                     bias=b1_ap(sp, 0))
                state[("hid", k)] = hid

            def hid_ev_dve(k):
                nb, sp = divmod(k, SM)
                ps = state[("psh", k)]
                hid = state[("hid", k)]
                nc.vector.tensor_scalar(hid[:, 1, :], ps[1][:], b1_ap(sp, 1),
                                        0.0, alu.add, alu.max)

            def fused_mms(k):
                nb, sp = divmod(k, SM)
                hid = state[("hid", k)]
                ps = psF.tile([P, 2, TB], F32, tag="psF")
                for oc in range(2):
                    for jc in range(2):
                        mm(ps[:, oc, :], w2_sb[:, sp, jc, oc, :],
                           hid[:, jc, :], start=(jc == 0), stop=(jc == 1))
                state[("psf", k)] = ps

            def ch_mms(k):
                nb, sp = divmod(k, SM)
                hid = state[("hid", k)]
                rqp = state[("rqp", nb)]
                ps = psF.tile([P, 2, TB], F32, tag="psF")
                for jc in range(2):
                    # identity matmul seeds the group with the shared rqp
                    mm(ps[:, jc, :], id_sb[:], rqp[:, jc, :],
                       start=True, stop=False)
                    for jci in range(2):
                        mm(ps[:, jc, :], wcf_sb[:, sp, jci, jc, :],
                           hid[:, jci, :], start=False, stop=(jci == 1))
                state[("psc", k)] = ps

            def fused_ev(k):
                nb, sp = divmod(k, SM)
                ps = state[("psf", k)]
                fsb = fpool.tile([P, 2, TB], BF16, tag="fsb")
                nc.scalar.activation(fsb[:, 0, :], ps[:, 0, :],
                                     AFT.Identity, bias=b2_ap(sp, 0))
                nc.vector.tensor_scalar_add(fsb[:, 1, :], ps[:, 1, :],
                                            b2_ap(sp, 1))
                state[("fsb", k)] = fsb

            def ch_relu(k):
                nb, sp = divmod(k, SM)
                ps = state[("psc", k)]
                ch = chpool.tile([P, 2, TB], BF16, tag="ch")
                nc.scalar.activation(ch[:, 0, :], ps[:, 0, :], AFT.Relu,
                                     bias=cb_ap(sp, 0))
                nc.vector.tensor_scalar(ch[:, 1, :], ps[:, 1, :],
                                        cb_ap(sp, 1), 0.0, alu.add, alu.max)
                state[("ch", k)] = ch

            def score_mms(k):
                ch = state[("ch", k)]
                ps = psS.tile([P, TB], F32, tag="psS")
                for jc in range(2):
                    mm(ps[:], wc2_sb[:, jc, :], ch[:, jc, :],
                       start=(jc == 0), stop=(jc == 1))
                state[("pss", k)] = ps

            def score_sig(k):
                ps = state[("pss", k)]
                sc = scpool.tile([P, TB], BF16, tag="sc")
                nc.scalar.activation(sc[:], ps[:], AFT.Sigmoid, bias=bc2_ap())
                state[("sc", k)] = sc

            def gating(k):
                nb, sp = divmod(k, SM)
                fsb = state[("fsb", k)]
                sc = state[("sc", k)]
                if sp == 0:
                    acc = accpool.tile([P, 2, TB], BF16, tag="acc")
                    state[("acc", nb)] = acc
                    for oc in range(2):
                        nc.vector.tensor_mul(acc[:, oc, :], fsb[:, oc, :], sc[:])
                else:
                    acc = state[("acc", nb)]
                    gf = gfpool.tile([P, 2, TB], BF16, tag="gf")
                    for oc in range(2):
                        nc.vector.tensor_mul(gf[:, oc, :], fsb[:, oc, :], sc[:])
                    nc.vector.tensor_add(acc[:], acc[:], gf[:])
                if sp == SM - 1:
                    for oc in range(2):
                        nc.sync.dma_start(outT[nb, oc], acc[:, oc, :])

            def rqp_mms(nb):
                rqt = rqts[nb]
                ps = psF.tile([P, 2, TB], F32, tag="psF")
                for jc in range(2):
                    for dc in range(2):
                        mm(ps[:, jc, :], wcq_sb[:, dc, jc, :],
                           rqt[:, dc, :], start=(dc == 0), stop=(dc == 1))
                state[("psr", nb)] = ps

            def rqp_ev(nb):
                # bf16: rqp re-enters the PE as rhs of the ch identity matmul
                ps = state[("psr", nb)]
                rqp = rqppool.tile([P, 2, TB], BF16, tag="rqp")
                nc.scalar.activation(rqp[:, 0, :], ps[:, 0, :], AFT.Identity)
                nc.vector.tensor_copy(rqp[:, 1, :], ps[:, 1, :])
                state[("rqp", nb)] = rqp

            def prefetch(nb):
                if nb >= NB or nb < len(xds):
                    return
                xdt = xpool.tile([P, SM, 2, TB], BF16, tag="xd")
                nc.gpsimd.dma_start(xdt[:], xd[nb])
                xds.append(xdt)
                x8t = x8pool.tile([P, M, 2, TB], FP8, tag="x8")
                nc.gpsimd.dma_start(x8t[:], x8[nb])
                x8s.append(x8t)
                rqt = rqpool.tile([P, 2, TB], BF16, tag="rq")
                nc.gpsimd.dma_start(rqt[:], rqT[nb])
                rqts.append(rqt)

            # ---- software-pipelined main loop ----
            # Per-engine queue order per slot:
            #   PE : rqp? | hid(a) | fused(b) | ch(b) | score(c)
            #   ACT: rqp0? | hidE0(a) | fusedE0(b) | sig(c) | chRelu0(b)
            #   DVE: rqp1? | gating(d) | hidE1(a) | fusedE1(b) | chRelu1(b)
            rqp_mms(0)
            rqp_ev(0)
            for s in range(NSLOT + 3):
                a, b, c, dd = s, s - 1, s - 2, s - 3
                if 0 < a < NSLOT and a % SM == 0:
                    prefetch(a // SM + 1)
                    rqp_mms(a // SM)
                    rqp_ev(a // SM)
                elif a == 0:
                    prefetch(1)
                # PE queue
                if a < NSLOT:
                    hid_mms(a)
                if 0 <= b < NSLOT:
                    fused_mms(b)
                    ch_mms(b)
                if 0 <= c < NSLOT:
                    score_mms(c)
                # evictions / elementwise
                if 0 <= dd < NSLOT:
                    gating(dd)
                if a < NSLOT:
                    hid_ev_act(a)
                    hid_ev_dve(a)
                if 0 <= b < NSLOT:
                    fused_ev(b)
                if 0 <= c < NSLOT:
                    score_sig(c)
                if 0 <= b < NSLOT:
                    ch_relu(b)
    return nc


def _get_nc():
    global _NC
    if _NC is None:
        _install_patches()
        _NC = _build_nc()
    return _NC


# ---------------------------------------------------------------------------
# host-side packing
# ---------------------------------------------------------------------------
def _pack_core(g, i, xTg, rqg, W1xT, G8h, G8l, W2g, WcfT, wcqp, wc2p, smg):
    mods = list(range(4 * g, 4 * g + 4))
    others = [t for t in range(M) if t not in mods]
    perm = mods + others
    bsl = slice(i * BC, (i + 1) * BC)
    # xd: own 4 modalities bf16 [nb, p, sp, dc, b]
    xdp = xTg[mods][:, :, bsl].reshape(SM, 2, P, NB, TB).transpose(3, 2, 0, 1, 4)
    xdp = np.ascontiguousarray(xdp).astype(NP_BF16)
    # x8: all 8 modalities (host-permuted) fp8*SX [nb, p, t, dc, b]
    x8p = (xTg[perm][:, :, bsl] * SX).reshape(M, 2, P, NB, TB) \
        .transpose(3, 2, 0, 1, 4)
    x8p = np.ascontiguousarray(x8p).astype(NP_FP8)
    rqp = rqg[:, bsl].reshape(2, P, NB, TB).transpose(2, 1, 0, 3)
    rqp = np.ascontiguousarray(rqp).astype(NP_BF16)
    return {
        "xd": xdp, "x8": x8p, "rqT": rqp, "w1x": W1xT[g], "g8h": G8h[g],
        "g8l": G8l[g], "w2w": W2g[g], "wcf": WcfT[g],
        "wcq": wcqp, "wc2r": wc2p, "smalls": smg[g],
        "ident": np.ascontiguousarray(np.eye(P, dtype=np.float32))
        .astype(NP_BF16),
    }


def kernel(x, reasoning_query, Wv, bv, Wo, bo, W1, b1, W2, b2,
           Wc1, bc1, wc2, bc2):
    f32 = np.float32
    x = np.asarray(x, dtype=f32)
    rq = np.asarray(reasoning_query, dtype=f32)
    Wv = np.asarray(Wv, dtype=f32)
    bv = np.asarray(bv, dtype=f32)
    Wo = np.asarray(Wo, dtype=f32)
    bo = np.asarray(bo, dtype=f32)
    W1 = np.asarray(W1, dtype=f32)
    b1 = np.asarray(b1, dtype=f32)
    W2 = np.asarray(W2, dtype=f32)
    b2 = np.asarray(b2, dtype=f32)
    Wc1 = np.asarray(Wc1, dtype=f32)
    bc1 = np.asarray(bc1, dtype=f32)
    wc2 = np.asarray(wc2, dtype=f32)
    bc2 = np.asarray(bc2, dtype=f32)

    nc = _get_nc()

    # ---- weight folding (host, weight-only preprocessing) ----
    W1x = W1[:, :, :D]                                   # [M, j, d]
    W1c = W1[:, :, D:] / 7.0                             # [M, j, e]
    # constant cross bias: c[s] = sum_{t!=s} bv[s,t]@Wo[s,t].T + bo[s,t]
    cfull = np.einsum("ste,stoe->sto", bv.astype(np.float64),
                      Wo.astype(np.float64)) + bo.astype(np.float64)
    for s in range(M):
        cfull[s, s] = 0.0
    c_all = cfull.sum(axis=1)                            # [M, D]
    b1eff = b1.astype(np.float64) + np.einsum(
        "so,sjo->sj", c_all / 7.0, W1.astype(np.float64)[:, :, D:])
    b1eff = b1eff.astype(f32)                            # [M, j]

    # G[s,t] = W1c[s] @ Wo[s,t] @ Wv[s,t]  (t != s)
    G = np.zeros((M, M, D, D), dtype=f32)
    for s in range(M):
        for t in range(M):
            if t != s:
                G[s, t] = W1c[s] @ (Wo[s, t] @ Wv[s, t])
    # Wcf2[s] = Wc1f @ W2[s]; cb[s] = bc1 + Wc1f @ b2[s]
    Wc1q, Wc1f = Wc1[:, :D], Wc1[:, D:]
    Wcf2 = np.einsum("jo,sod->sjd", Wc1f, W2)            # [M, j, d(hid j)]
    cb = bc1[None, :] + b2 @ Wc1f.T                      # [M, j]

    # ---- pack weights per modality group ----
    # The hid PSUM runs at scale SH (fp8 operand scales SX*SG); the diag bf16
    # weights carry SH, and the post-hid weights divide it back out.
    W1xT, G8h, G8l, W2T, WcfT, smg = [], [], [], [], [], []
    for g in range(2):
        mods = list(range(4 * g, 4 * g + 4))
        others = [t for t in range(M) if t not in mods]
        perm = mods + others
        # W1x lhsT (*SH): [p(d), sp, dc, jc, j']
        w1b = (W1x[mods] * SH).reshape(SM, 2, P, 2, P).transpose(4, 0, 3, 1, 2)
        W1xT.append(np.ascontiguousarray(w1b).astype(NP_BF16))
        # G fp8 hi/lo lhsT (*SG): [p(d), sp, ti, dc, jc, j']
        gb = np.empty((SM, M - 1, D, D), dtype=f32)
        for sp in range(SM):
            tlist = [t for t in range(M) if t != sp]
            for ti, tslot in enumerate(tlist):
                gb[sp, ti] = G[mods[sp], perm[tslot]] * SG
        gb = gb.reshape(SM, M - 1, 2, P, 2, P).transpose(5, 0, 1, 4, 2, 3)
        gb = np.ascontiguousarray(gb)
        ghi = gb.astype(NP_FP8)
        glo = (gb - ghi.astype(f32)).astype(NP_FP8)
        G8h.append(ghi)
        G8l.append(glo)
        # W2 lhsT: 1/M output mean and 1/SH hid scale folded in
        w2b = (W2[mods] / (M * SH)).reshape(SM, 2, P, 2, P) \
            .transpose(4, 0, 3, 1, 2)
        W2T.append(np.ascontiguousarray(w2b).astype(NP_BF16))
        # Wcf2 lhsT (/SH): [p(j_in), sp, jc_in, jc_out, j'']
        wcb = (Wcf2[mods] / SH).reshape(SM, 2, P, 2, P).transpose(4, 0, 3, 1, 2)
        WcfT.append(np.ascontiguousarray(wcb).astype(NP_BF16))
        sm = np.zeros((P, 25), dtype=f32)
        sm[:, 0:8] = (b1eff[mods] * SH).reshape(SM, 2, P) \
            .transpose(2, 0, 1).reshape(P, 8)
        sm[:, 8:16] = (b2[mods] / M).reshape(SM, 2, P) \
            .transpose(2, 0, 1).reshape(P, 8)
        sm[:, 16:24] = cb[mods].reshape(SM, 2, P).transpose(2, 0, 1).reshape(P, 8)
        sm[:, 24] = bc2.reshape(-1)[0]
        smg.append(sm)
    # Wc1q lhsT: [p(d), dc, jc, j']
    wcqp = Wc1q.reshape(2, P, 2, P).transpose(3, 2, 0, 1)
    wcqp = np.ascontiguousarray(wcqp).astype(NP_BF16)
    # wc2 column-replicated: [p(j), jc, col]
    wc2p = np.ascontiguousarray(
        np.broadcast_to(wc2.reshape(2, P).T[:, :, None], (P, 2, P))
    ).astype(NP_BF16)

    xTg = np.ascontiguousarray(x.transpose(0, 2, 1))     # [8, 256, B]
    rqg = np.ascontiguousarray(rq.T)                     # [256, B]

    in_maps = []
    for core in range(8):
        g, i = core // 4, core % 4
        in_maps.append(_pack_core(g, i, xTg, rqg, W1xT, G8h, G8l, W2T, WcfT,
                                  wcqp, wc2p, smg))

    if TRACE:
        _install_ntff_hook()
    res = run_bass_kernel_spmd(nc, in_maps, list(range(8)), trace=TRACE)
    LAST["exec_time_ns"] = res.exec_time_ns
    LAST["res"] = res

    out = np.empty((B, D), dtype=f32)
    for i in range(4):
        part = res.results[i]["outT"].astype(f32) + \
            res.results[i + 4]["outT"].astype(f32)       # [NB, 2, P, TB]
        blk = part.transpose(0, 3, 1, 2).reshape(BC, D)  # [BC, 256]
        out[i * BC:(i + 1) * BC] = blk
    return out


# revision 39
# speedup vs baseline: 1.3155x; 1.3155x over previous
"""Trainium2 Bass kernel for nn_CrossModalAttention (M=8, D=256, B=8192).

Math restructuring (seq_len=1 MHA => out_proj(V_proj(x_t)) per (s,t) pair):
  hid[s]   = relu( W1x[s] @ x_s + sum_{t!=s} G[s,t] @ x_t + b1eff[s] )
  fused[s] = W2[s] @ hid[s] + b2[s]
  ch[s]    = relu( Wc1q @ rq + Wcf2[s] @ hid[s] + cb[s] )   Wcf2 = Wc1f @ W2[s]
  score[s] = sigmoid(wc2 . ch[s] + bc2)
  out      = mean_s fused[s] * score[s]
where G[s,t] = (W1c[s]/7) @ Wo[s,t] @ Wv[s,t] is folded on the HOST
(weight-only preprocessing), so the device runs a single dense pipeline.

Sharding: 8 cores = 4 batch shards x 2 modality groups. Core (g, i) handles
source modalities [4g..4g+3] for batch rows [i*2048, (i+1)*2048). All
activations are feature-major [feature, batch] in SBUF; weights ship as
pre-transposed bf16 lhsT blocks. The device loop is software-pipelined over
source modalities so the PE issues matmuls back-to-back while ACT/DVE handle
evictions in the shadow.
"""

import os
import sys
import types

import numpy as np
import ml_dtypes

# ---------------------------------------------------------------------------
# environment / concourse import
# ---------------------------------------------------------------------------
try:
    import concourse.bass as bass
except ImportError:  # pragma: no cover
    for p in ("/opt/trn_rl_repo", "/root/.axon_site/_ro/trn_rl_repo"):
        if os.path.isdir(p) and p not in sys.path:
            sys.path.insert(0, p)
    import concourse.bass as bass

import concourse.mybir as mybir
import concourse.tile as tile
from concourse.bass_utils import run_bass_kernel_spmd
from concourse.tile_sem_assignment import N_PROCS
from concourse.vector_clock import ScopedClock, VectorClock

F32 = mybir.dt.float32
BF16 = mybir.dt.bfloat16
FP8 = mybir.dt.float8e4
NP_BF16 = ml_dtypes.bfloat16
NP_FP8 = ml_dtypes.float8_e4m3
AFT = mybir.ActivationFunctionType
DR = mybir.MatmulPerfMode.DoubleRow

SX = 16.0      # fp8 scale on x
SG = 256.0     # fp8 scale on G
SH = SX * SG   # PSUM scale of the hid accumulation

# module-level knobs (test.py pokes these)
TRACE = False
USE_F32R = True  # unused; kept for test.py compat
LAST = {}

P = 128          # partitions
M = 8            # modalities
D = 256          # embedding dim
B = 8192         # batch
SM = 4           # source modalities per core
NB = 4           # batch tiles per core
TB = 512         # batch tile size (per-core batch = NB*TB = 2048)
BC = NB * TB

_MAX_WAITS = 1   # this walrus build supports one sync-wait per instruction


# ---------------------------------------------------------------------------
# walrus single-wait workaround: split multi-wait instructions
# ---------------------------------------------------------------------------
def _patched_drain_and_barrier(self, tick_clock, wait_clock):
    gc = tick_clock.global_clock
    for p in range(N_PROCS):
        t = gc[p]
        if t <= 0:
            continue
        sub = VectorClock([t if q == p else 0 for q in range(N_PROCS)])
        nop_inst = self.nc.sync.nop(nofuse=True)
        wait_clock.add_sem_waits(nop_inst.ins, ScopedClock({None: sub}))
    self.nc.sync.drain()
    self.nc.all_engine_barrier()
    assert self.sems is not None
    popped = self.nc._tile_sem_poison_stack.pop()
    assert popped is self._sem_poison
    self.nc.clear_and_free_semaphores(list(self.sems.allocated().values()))
    self.nc.all_engine_barrier()


_orig_commit_and_lower = None


def _patched_commit_and_lower(self, inst, original_block, old_bb_map, bb_to_exit_bb):
    si = getattr(inst, "sync_info", None)
    if (
        si is not None
        and si.on_wait
        and len(si.on_wait) > _MAX_WAITS
        and inst.engine != mybir.EngineType.Unassigned
    ):
        waits = list(si.on_wait)
        keep = waits[-_MAX_WAITS:]
        for w in waits[:-_MAX_WAITS]:
            nop = mybir.InstNoOp(
                name=self.nc.get_next_instruction_name(),
                sync_info=mybir.SyncInfo(on_wait=[w], on_update=[]),
                bass_nofuse=True,
                engine=inst.engine,
            )
            self._commit_instruction(nop)
        inst.sync_info = mybir.SyncInfo(on_wait=keep, on_update=list(si.on_update))
    return _orig_commit_and_lower(self, inst, original_block, old_bb_map, bb_to_exit_bb)


def _install_patches():
    global _orig_commit_and_lower
    if _orig_commit_and_lower is None:
        _orig_commit_and_lower = tile.TileContext._commit_and_lower
        tile.TileContext._drain_and_barrier = _patched_drain_and_barrier
        tile.TileContext._commit_and_lower = _patched_commit_and_lower


# ---------------------------------------------------------------------------
# optional NTFF profile hook (for HW exec-time measurement; safe no-op on fail)
# ---------------------------------------------------------------------------
def _install_ntff_hook():
    try:
        import antenv

        if "antenv.axon_hooks" in sys.modules:
            return True
        mod = types.ModuleType("antenv.axon_hooks")
        mod._hook = None
        mod.set_axon_ntff_profile_hook = lambda h: setattr(mod, "_hook", h)
        mod.get_axon_ntff_profile_hook = lambda: mod._hook
        sys.modules["antenv.axon_hooks"] = mod
        antenv.axon_hooks = mod
        from trn_agent_boot.trn_boot import _ntff_profile_via_ctypes

        hook = _ntff_profile_via_ctypes("/opt/axon/libaxon_pjrt.so")
        mod.set_axon_ntff_profile_hook(hook)
        return hook is not None
    except Exception:
        return False


# ---------------------------------------------------------------------------
# device program
# ---------------------------------------------------------------------------
_NC = None


def _build_nc():
    nc = bass.Bass()

    # inputs (per-core shards; same shapes on every core)
    # xd: bf16 x of the core's own 4 source modalities (diagonal matmuls)
    xd = nc.dram_tensor("xd", [NB, P, SM, 2, TB], BF16, kind="ExternalInput")
    # x8: fp8(e4m3) x of all 8 modalities, scaled by SX (cross matmuls)
    x8 = nc.dram_tensor("x8", [NB, P, M, 2, TB], FP8, kind="ExternalInput")
    rqT = nc.dram_tensor("rqT", [NB, P, 2, TB], BF16, kind="ExternalInput")
    # diag hid weights (W1x * SH): [p(d-in-chunk), sp, dc, jc, j']
    w1x = nc.dram_tensor("w1x", [P, SM, 2, 2, P], BF16, kind="ExternalInput")
    # cross hid weights G*SG as fp8: [p(d), sp, ti, dc, jc, j']
    g8h = nc.dram_tensor("g8h", [P, SM, M - 1, 2, 2, P], FP8,
                         kind="ExternalInput")
    # fused weights: [p(j-in-chunk), sp, jc, oc, o']
    w2w = nc.dram_tensor("w2w", [P, SM, 2, 2, P], BF16, kind="ExternalInput")
    # controller hid weights (Wc1f@W2): [p(j-in-chunk), sp, jc_in, jc_out, j'']
    wcf = nc.dram_tensor("wcf", [P, SM, 2, 2, P], BF16, kind="ExternalInput")
    # controller query weights: [p(d-in-chunk), dc, jc, j']
    wcq = nc.dram_tensor("wcq", [P, 2, 2, P], BF16, kind="ExternalInput")
    # column-replicated wc2: [p(j-in-chunk), jc, col]
    wc2r = nc.dram_tensor("wc2r", [P, 2, P], BF16, kind="ExternalInput")
    # f32 per-partition constants: [:,0:8] b1eff (sp,jc), [:,8:16] b2 (sp,oc),
    # [:,16:24] cb (sp,jc), [:,24] bc2
    smalls = nc.dram_tensor("smalls", [P, 25], F32, kind="ExternalInput")
    # identity matrix (lhsT) used to add rqp into the ch PSUM group
    ident = nc.dram_tensor("ident", [P, P], BF16, kind="ExternalInput")
    outT = nc.dram_tensor("outT", [NB, 2, P, TB], BF16, kind="ExternalOutput")

    mm = nc.tensor.matmul
    alu = mybir.AluOpType

    with tile.TileContext(nc) as tc:
        with (
            tc.tile_pool(name="const", bufs=1) as cpool,
            tc.tile_pool(name="xpool", bufs=2) as xpool,
            tc.tile_pool(name="x8pool", bufs=2) as x8pool,
            tc.tile_pool(name="rqpool", bufs=2) as rqpool,
            tc.tile_pool(name="rqppool", bufs=2) as rqppool,
            tc.tile_pool(name="hidpool", bufs=3) as hidpool,
            tc.tile_pool(name="fpool", bufs=2) as fpool,
            tc.tile_pool(name="chpool", bufs=2) as chpool,
            tc.tile_pool(name="scpool", bufs=2) as scpool,
            tc.tile_pool(name="gfpool", bufs=2) as gfpool,
            tc.tile_pool(name="accpool", bufs=2) as accpool,
            tc.tile_pool(name="psH", bufs=3, space="PSUM") as psH,
            tc.tile_pool(name="psF", bufs=2, space="PSUM") as psF,
            tc.tile_pool(name="psS", bufs=1, space="PSUM") as psS,
        ):
            # ---- resident constants (spread across the three DMA-capable
            # queues — sync, scalar, gpsimd — so startup transfers run on
            # parallel DMA engines, ordered by first use) ----
            wcq_sb = cpool.tile([P, 2, 2, P], BF16, tag="wcq")
            nc.sync.dma_start(wcq_sb[:], wcq[:])
            rqt0 = rqpool.tile([P, 2, TB], BF16, tag="rq")
            nc.sync.dma_start(rqt0[:], rqT[0])
            sm_sb = cpool.tile([P, 25], F32, tag="smalls")
            nc.sync.dma_start(sm_sb[:], smalls[:])

            x80 = x8pool.tile([P, M, 2, TB], FP8, tag="x8")
            nc.gpsimd.dma_start(x80[:], x8[0])
            xd0 = xpool.tile([P, SM, 2, TB], BF16, tag="xd")
            nc.gpsimd.dma_start(xd0[:], xd[0])

            w1x_sb = cpool.tile([P, SM, 2, 2, P], BF16, tag="w1x")
            nc.scalar.dma_start(w1x_sb[:], w1x[:])
            g8h_sb = cpool.tile([P, SM, M - 1, 2, 2, P], FP8, tag="g8h")
            nc.scalar.dma_start(g8h_sb[:, 0], g8h[:, 0])
            nc.scalar.dma_start(g8h_sb[:, 1], g8h[:, 1])

            w2_sb = cpool.tile([P, SM, 2, 2, P], BF16, tag="w2w")
            nc.sync.dma_start(w2_sb[:], w2w[:])
            wcf_sb = cpool.tile([P, SM, 2, 2, P], BF16, tag="wcf")
            nc.sync.dma_start(wcf_sb[:], wcf[:])
            id_sb = cpool.tile([P, P], BF16, tag="ident")
            nc.sync.dma_start(id_sb[:], ident[:])
            wc2_sb = cpool.tile([P, 2, P], BF16, tag="wc2r")
            nc.sync.dma_start(wc2_sb[:], wc2r[:])
            nc.scalar.dma_start(g8h_sb[:, 2], g8h[:, 2])
            nc.scalar.dma_start(g8h_sb[:, 3], g8h[:, 3])

            # PE warm-up: dummy matmuls on the first-arriving tile keep the
            # tensor engine busy through the startup DMA window so the HAM
            # clock ramps to 2.4 GHz before real work starts.
            ps_warm = psS.tile([P, TB], F32, tag="psS", name="ps_warm")

            def warmup(n):
                for w in range(n):
                    mm(ps_warm[:], wcq_sb[:, 0, 0, :], wcq_sb[:],
                       start=True, stop=True, skip_group_check=True)

            warmup(28)

            def b1_ap(sp, jc):
                return sm_sb[:, sp * 2 + jc:sp * 2 + jc + 1]

            def b2_ap(sp, oc):
                return sm_sb[:, 8 + sp * 2 + oc:8 + sp * 2 + oc + 1]

            def cb_ap(sp, jc):
                return sm_sb[:, 16 + sp * 2 + jc:16 + sp * 2 + jc + 1]

            def bc2_ap():
                return sm_sb[:, 24:25]

            NSLOT = NB * SM
            xds = [xd0]
            x8s = [x80]
            rqts = [rqt0]
            state = {}

            def hid_mms(k):
                # x8 slots are host-permuted to [own 4 mods, other 4], so the
                # cross-target slots for local source sp are all t != sp,
                # identically on every core.
                nb, sp = divmod(k, SM)
                xdt = xds[nb]
                x8t = x8s[nb]
                ps = [psH.tile([P, TB], F32, tag="psH", name=f"psh{k}_{j}")
                      for j in range(2)]
                tlist = [t for t in range(M) if t != sp]
                for jc in range(2):
                    # diagonal W1x (bf16, pre-scaled by SH)
                    for dc in range(2):
                        mm(ps[jc][:], w1x_sb[:, sp, dc, jc, :],
                           xdt[:, sp, dc, :], start=(dc == 0), stop=False)
                    # cross G (fp8, DoubleRow over the dc pair)
                    for ti, t in enumerate(tlist):
                        mm(ps[jc][:], g8h_sb[:, sp, ti, :, jc, :],
                           x8t[:, t, :, :], start=False,
                           stop=(ti == M - 2), perf_mode=DR,
                           skip_group_check=True)
                state[("psh", k)] = ps

            def hid_ev_act(k):
                nb, sp = divmod(k, SM)
                ps = state[("psh", k)]
                hid = hidpool.tile([P, 2, TB], BF16, tag="hid")
                nc.scalar.activation(hid[:, 0, :], ps[0][:], AFT.Relu,
                                     bias=b1_ap(sp, 0))
                state[("hid", k)] = hid

            def hid_ev_dve(k):
                nb, sp = divmod(k, SM)
                ps = state[("psh", k)]
                hid = state[("hid", k)]
                nc.vector.tensor_scalar(hid[:, 1, :], ps[1][:], b1_ap(sp, 1),
                                        0.0, alu.add, alu.max)

            def fused_mms(k):
                nb, sp = divmod(k, SM)
                hid = state[("hid", k)]
                ps = psF.tile([P, 2, TB], F32, tag="psF")
                for oc in range(2):
                    for jc in range(2):
                        mm(ps[:, oc, :], w2_sb[:, sp, jc, oc, :],
                           hid[:, jc, :], start=(jc == 0), stop=(jc == 1))
                state[("psf", k)] = ps

            def ch_mms(k):
                nb, sp = divmod(k, SM)
                hid = state[("hid", k)]
                rqp = state[("rqp", nb)]
                ps = psF.tile([P, 2, TB], F32, tag="psF")
                for jc in range(2):
                    # identity matmul seeds the group with the shared rqp
                    mm(ps[:, jc, :], id_sb[:], rqp[:, jc, :],
                       start=True, stop=False)
                    for jci in range(2):
                        mm(ps[:, jc, :], wcf_sb[:, sp, jci, jc, :],
                           hid[:, jci, :], start=False, stop=(jci == 1))
                state[("psc", k)] = ps

            def fused_ev(k):
                nb, sp = divmod(k, SM)
                ps = state[("psf", k)]
                fsb = fpool.tile([P, 2, TB], BF16, tag="fsb")
                nc.scalar.activation(fsb[:, 0, :], ps[:, 0, :],
                                     AFT.Identity, bias=b2_ap(sp, 0))
                nc.vector.tensor_scalar_add(fsb[:, 1, :], ps[:, 1, :],
                                            b2_ap(sp, 1))
                state[("fsb", k)] = fsb

            def ch_relu(k):
                nb, sp = divmod(k, SM)
                ps = state[("psc", k)]
                ch = chpool.tile([P, 2, TB], BF16, tag="ch")
                nc.scalar.activation(ch[:, 0, :], ps[:, 0, :], AFT.Relu,
                                     bias=cb_ap(sp, 0))
                nc.vector.tensor_scalar(ch[:, 1, :], ps[:, 1, :],
                                        cb_ap(sp, 1), 0.0, alu.add, alu.max)
                state[("ch", k)] = ch

            def score_mms(k):
                ch = state[("ch", k)]
                ps = psS.tile([P, TB], F32, tag="psS")
                for jc in range(2):
                    mm(ps[:], wc2_sb[:, jc, :], ch[:, jc, :],
                       start=(jc == 0), stop=(jc == 1))
                state[("pss", k)] = ps

            def score_sig(k):
                ps = state[("pss", k)]
                sc = scpool.tile([P, TB], BF16, tag="sc")
                nc.scalar.activation(sc[:], ps[:], AFT.Sigmoid, bias=bc2_ap())
                state[("sc", k)] = sc

            def gating(k):
                nb, sp = divmod(k, SM)
                fsb = state[("fsb", k)]
                sc = state[("sc", k)]
                if sp == 0:
                    acc = accpool.tile([P, 2, TB], BF16, tag="acc")
                    state[("acc", nb)] = acc
                    for oc in range(2):
                        nc.vector.tensor_mul(acc[:, oc, :], fsb[:, oc, :], sc[:])
                else:
                    acc = state[("acc", nb)]
                    gf = gfpool.tile([P, 2, TB], BF16, tag="gf")
                    for oc in range(2):
                        nc.vector.tensor_mul(gf[:, oc, :], fsb[:, oc, :], sc[:])
                    nc.vector.tensor_add(acc[:], acc[:], gf[:])
                if sp == SM - 1:
                    for oc in range(2):
                        nc.sync.dma_start(outT[nb, oc], acc[:, oc, :])

            def rqp_mms(nb):
                rqt = rqts[nb]
                ps = psF.tile([P, 2, TB], F32, tag="psF")
                for jc in range(2):
                    for dc in range(2):
                        mm(ps[:, jc, :], wcq_sb[:, dc, jc, :],
                           rqt[:, dc, :], start=(dc == 0), stop=(dc == 1))
                state[("psr", nb)] = ps

            def rqp_ev(nb):
                # bf16: rqp re-enters the PE as rhs of the ch identity matmul
                ps = state[("psr", nb)]
                rqp = rqppool.tile([P, 2, TB], BF16, tag="rqp")
                nc.scalar.activation(rqp[:, 0, :], ps[:, 0, :], AFT.Identity)
                nc.vector.tensor_copy(rqp[:, 1, :], ps[:, 1, :])
                state[("rqp", nb)] = rqp

            def prefetch(nb):
                if nb >= NB or nb < len(xds):
                    return
                xdt = xpool.tile([P, SM, 2, TB], BF16, tag="xd")
                nc.gpsimd.dma_start(xdt[:], xd[nb])
                xds.append(xdt)
                x8t = x8pool.tile([P, M, 2, TB], FP8, tag="x8")
                nc.gpsimd.dma_start(x8t[:], x8[nb])
                x8s.append(x8t)
                rqt = rqpool.tile([P, 2, TB], BF16, tag="rq")
                nc.gpsimd.dma_start(rqt[:], rqT[nb])
                rqts.append(rqt)

            # ---- software-pipelined main loop ----
            # Per-engine queue order per slot:
            #   PE : rqp? | hid(a) | fused(b) | ch(b) | score(c)
            #   ACT: rqp0? | hidE0(a) | fusedE0(b) | sig(c) | chRelu0(b)
            #   DVE: rqp1? | gating(d) | hidE1(a) | fusedE1(b) | chRelu1(b)
            rqp_mms(0)
            rqp_ev(0)
            warmup(20)
            for s in range(NSLOT + 3):
                a, b, c, dd = s, s - 1, s - 2, s - 3
                if 0 < a < NSLOT and a % SM == 0:
                    prefetch(a // SM + 1)
                    rqp_mms(a // SM)
                    rqp_ev(a // SM)
                elif a == 0:
                    prefetch(1)
                # PE queue
                if a < NSLOT:
                    hid_mms(a)
                if 0 <= b < NSLOT:
                    fused_mms(b)
                    ch_mms(b)
                if 0 <= c < NSLOT:
                    score_mms(c)
                # evictions / elementwise
                if 0 <= dd < NSLOT:
                    gating(dd)
                if a < NSLOT:
                    hid_ev_act(a)
                    hid_ev_dve(a)
                if 0 <= b < NSLOT:
                    fused_ev(b)
                if 0 <= c < NSLOT:
                    score_sig(c)
                if 0 <= b < NSLOT:
                    ch_relu(b)
    return nc


def _get_nc():
    global _NC
    if _NC is None:
        _install_patches()
        _NC = _build_nc()
    return _NC


# ---------------------------------------------------------------------------
# host-side packing
# ---------------------------------------------------------------------------
def _pack_core(g, i, xTg, rqg, W1xT, G8h, W2g, WcfT, wcqp, wc2p, smg):
    mods = list(range(4 * g, 4 * g + 4))
    others = [t for t in range(M) if t not in mods]
    perm = mods + others
    bsl = slice(i * BC, (i + 1) * BC)
    # xd: own 4 modalities bf16 [nb, p, sp, dc, b]
    xdp = xTg[mods][:, :, bsl].reshape(SM, 2, P, NB, TB).transpose(3, 2, 0, 1, 4)
    xdp = np.ascontiguousarray(xdp).astype(NP_BF16)
    # x8: all 8 modalities (host-permuted) fp8*SX [nb, p, t, dc, b]
    x8p = (xTg[perm][:, :, bsl] * SX).reshape(M, 2, P, NB, TB) \
        .transpose(3, 2, 0, 1, 4)
    x8p = np.ascontiguousarray(x8p).astype(NP_FP8)
    rqp = rqg[:, bsl].reshape(2, P, NB, TB).transpose(2, 1, 0, 3)
    rqp = np.ascontiguousarray(rqp).astype(NP_BF16)
    return {
        "xd": xdp, "x8": x8p, "rqT": rqp, "w1x": W1xT[g], "g8h": G8h[g],
        "w2w": W2g[g], "wcf": WcfT[g],
        "wcq": wcqp, "wc2r": wc2p, "smalls": smg[g],
        "ident": np.ascontiguousarray(np.eye(P, dtype=np.float32))
        .astype(NP_BF16),
    }


def kernel(x, reasoning_query, Wv, bv, Wo, bo, W1, b1, W2, b2,
           Wc1, bc1, wc2, bc2):
    f32 = np.float32
    x = np.asarray(x, dtype=f32)
    rq = np.asarray(reasoning_query, dtype=f32)
    Wv = np.asarray(Wv, dtype=f32)
    bv = np.asarray(bv, dtype=f32)
    Wo = np.asarray(Wo, dtype=f32)
    bo = np.asarray(bo, dtype=f32)
    W1 = np.asarray(W1, dtype=f32)
    b1 = np.asarray(b1, dtype=f32)
    W2 = np.asarray(W2, dtype=f32)
    b2 = np.asarray(b2, dtype=f32)
    Wc1 = np.asarray(Wc1, dtype=f32)
    bc1 = np.asarray(bc1, dtype=f32)
    wc2 = np.asarray(wc2, dtype=f32)
    bc2 = np.asarray(bc2, dtype=f32)

    nc = _get_nc()

    # ---- weight folding (host, weight-only preprocessing) ----
    W1x = W1[:, :, :D]                                   # [M, j, d]
    W1c = W1[:, :, D:] / 7.0                             # [M, j, e]
    # constant cross bias: c[s] = sum_{t!=s} bv[s,t]@Wo[s,t].T + bo[s,t]
    cfull = np.einsum("ste,stoe->sto", bv.astype(np.float64),
                      Wo.astype(np.float64)) + bo.astype(np.float64)
    for s in range(M):
        cfull[s, s] = 0.0
    c_all = cfull.sum(axis=1)                            # [M, D]
    b1eff = b1.astype(np.float64) + np.einsum(
        "so,sjo->sj", c_all / 7.0, W1.astype(np.float64)[:, :, D:])
    b1eff = b1eff.astype(f32)                            # [M, j]

    # G[s,t] = W1c[s] @ Wo[s,t] @ Wv[s,t]  (t != s)
    G = np.zeros((M, M, D, D), dtype=f32)
    for s in range(M):
        for t in range(M):
            if t != s:
                G[s, t] = W1c[s] @ (Wo[s, t] @ Wv[s, t])
    # Wcf2[s] = Wc1f @ W2[s]; cb[s] = bc1 + Wc1f @ b2[s]
    Wc1q, Wc1f = Wc1[:, :D], Wc1[:, D:]
    Wcf2 = np.einsum("jo,sod->sjd", Wc1f, W2)            # [M, j, d(hid j)]
    cb = bc1[None, :] + b2 @ Wc1f.T                      # [M, j]

    # ---- pack weights per modality group ----
    # The hid PSUM runs at scale SH (fp8 operand scales SX*SG); the diag bf16
    # weights carry SH, and the post-hid weights divide it back out.
    W1xT, G8h, W2T, WcfT, smg = [], [], [], [], []
    for g in range(2):
        mods = list(range(4 * g, 4 * g + 4))
        others = [t for t in range(M) if t not in mods]
        perm = mods + others
        # W1x lhsT (*SH): [p(d), sp, dc, jc, j']
        w1b = (W1x[mods] * SH).reshape(SM, 2, P, 2, P).transpose(4, 0, 3, 1, 2)
        W1xT.append(np.ascontiguousarray(w1b).astype(NP_BF16))
        # G fp8 hi/lo lhsT (*SG): [p(d), sp, ti, dc, jc, j']
        gb = np.empty((SM, M - 1, D, D), dtype=f32)
        for sp in range(SM):
            tlist = [t for t in range(M) if t != sp]
            for ti, tslot in enumerate(tlist):
                gb[sp, ti] = G[mods[sp], perm[tslot]] * SG
        gb = gb.reshape(SM, M - 1, 2, P, 2, P).transpose(5, 0, 1, 4, 2, 3)
        gb = np.ascontiguousarray(gb)
        G8h.append(gb.astype(NP_FP8))
        # W2 lhsT: 1/M output mean and 1/SH hid scale folded in
        w2b = (W2[mods] / (M * SH)).reshape(SM, 2, P, 2, P) \
            .transpose(4, 0, 3, 1, 2)
        W2T.append(np.ascontiguousarray(w2b).astype(NP_BF16))
        # Wcf2 lhsT (/SH): [p(j_in), sp, jc_in, jc_out, j'']
        wcb = (Wcf2[mods] / SH).reshape(SM, 2, P, 2, P).transpose(4, 0, 3, 1, 2)
        WcfT.append(np.ascontiguousarray(wcb).astype(NP_BF16))
        sm = np.zeros((P, 25), dtype=f32)
        sm[:, 0:8] = (b1eff[mods] * SH).reshape(SM, 2, P) \
            .transpose(2, 0, 1).reshape(P, 8)
        sm[:, 8:16] = (b2[mods] / M).reshape(SM, 2, P) \
            .transpose(2, 0, 1).reshape(P, 8)
        sm[:, 16:24] = cb[mods].reshape(SM, 2, P).transpose(2, 0, 1).reshape(P, 8)
        sm[:, 24] = bc2.reshape(-1)[0]
        smg.append(sm)
    # Wc1q lhsT: [p(d), dc, jc, j']
    wcqp = Wc1q.reshape(2, P, 2, P).transpose(3, 2, 0, 1)
    wcqp = np.ascontiguousarray(wcqp).astype(NP_BF16)
    # wc2 column-replicated: [p(j), jc, col]
    wc2p = np.ascontiguousarray(
        np.broadcast_to(wc2.reshape(2, P).T[:, :, None], (P, 2, P))
    ).astype(NP_BF16)

    xTg = np.ascontiguousarray(x.transpose(0, 2, 1))     # [8, 256, B]
    rqg = np.ascontiguousarray(rq.T)                     # [256, B]

    in_maps = []
    for core in range(8):
        g, i = core // 4, core % 4
        in_maps.append(_pack_core(g, i, xTg, rqg, W1xT, G8h, W2T, WcfT,
                                  wcqp, wc2p, smg))

    if TRACE:
        _install_ntff_hook()
    res = run_bass_kernel_spmd(nc, in_maps, list(range(8)), trace=TRACE)
    LAST["exec_time_ns"] = res.exec_time_ns
    LAST["res"] = res

    out = np.empty((B, D), dtype=f32)
    for i in range(4):
        part = res.results[i]["outT"].astype(f32) + \
            res.results[i + 4]["outT"].astype(f32)       # [NB, 2, P, TB]
        blk = part.transpose(0, 3, 1, 2).reshape(BC, D)  # [BC, 256]
        out[i * BC:(i + 1) * BC] = blk
    return out
